# revision 12
# baseline (speedup 1.0000x reference)
"""Trainium2 Bass kernel for nn_BasicLSTM (single-step LSTM cell variant).

Reference computation (B=4096, D=1024, S=1024):
    pre_f = inputs @ w_f + h0 @ u_f + b_f
    f     = sigmoid(pre_f)
    i     = sigmoid(inputs @ w_i + h0 @ u_i + b_i)
    o     = sigmoid(inputs @ w_o + h0 @ u_o + b_o)
    c_new = f * c0 + f * i          (input_cell reuses the forget gate)
    h_new = o * tanh(c_new)
    returns (h_new, c_new)

Sharding: batch 4-way x state 2-way over 8 NeuronCores. Core c handles
batch rows [ (c//2)*1024 : (c//2+1)*1024 ) and state cols
[ (c%2)*512 : (c%2+1)*512 ). Host-side prep per core:
    xt  = concat([inputs_rows, h0_rows], 1).T           # [2048, 1024] fp16
    w_g = concat([w_g[:, cols], u_g[:, cols]], 0)       # [2048, 512]  fp16
so the device kernel is three plain matmuls (K=2048 contraction on the
partition axis) plus fused elementwise, no on-device transposes.

Performance notes (the 384-matmul stream runs at the PE roofline of
~216 ns per N=512 fp16 matmul; everything else is arranged around it):
  - DMA instructions cost ~0.6 us of issue time each and the two HWDGE
    rings (sync, scalar) round-robin SDMA bandwidth per packet. The
    phase-f stream is split: xt tiles (2 KB rows) on sync, wf tiles
    (1 KB rows) on scalar -- the 2:1 byte ratio matches the per-packet
    round-robin so each (xt_k, wf_k) pair lands together, ahead of the
    PE's 1.73 us/k-step consumption.
  - Remaining inputs ride the sync ring behind the xt stream, ordered
    by first use: b_f, wi (4 chunks, consumed k-major), c0, b_i, wo,
    b_o. Phases f and i are k-major so weight chunks stream.
  - The HAM clock gate keeps the PE at 1.2 GHz until ~3.4 us of busy;
    four dummy matmuls on a memset tile fill the initial DMA wait.
  - Bias adds run on GpSimd, sig/tanh on Scalar, muls on Vector, so no
    engine queue serializes the phase-boundary chains.
  - c_new is independent of the o-gate: co is computed and stored while
    phase o runs. The tail after the last matmul is bias+sig+mul+store
    of one h tile, split in halves across the two DMA rings.
  - c0 / ho / co move as fp16 (host converts), halving that traffic.
"""

import sys

sys.path.insert(0, "/opt/trn_rl_repo")

import numpy as np

B, D, S = 4096, 1024, 1024
N_CORES = 8
BB, SB = 4, 2          # batch blocks x state blocks
B_CORE = B // BB       # 1024 rows per core
S_CORE = S // SB       # 512 state cols per core
K = D + S              # 2048 contraction
KT = K // 128          # 16 k-tiles
BT = B_CORE // 128     # 8 batch tiles per core
WCH = 4                # wi DMA chunks (k-tiles each)

_CACHE: dict = {}


def _build_nc():
    import concourse.mybir as mybir
    import concourse.tile as tile
    from concourse import bacc

    f32 = mybir.dt.float32
    f16 = mybir.dt.float16

    nc = bacc.Bacc("TRN2", target_bir_lowering=False, debug=False,
                   num_devices=N_CORES)

    xt = nc.dram_tensor("xt", [K, B_CORE], f16, kind="ExternalInput")
    w = {g: nc.dram_tensor(f"w{g}", [K, S_CORE], f16, kind="ExternalInput")
         for g in "fio"}
    bias = {g: nc.dram_tensor(f"b{g}", [128, S_CORE], f32,
                              kind="ExternalInput") for g in "fio"}
    c0 = nc.dram_tensor("c0", [B_CORE, S_CORE], f16, kind="ExternalInput")
    ho = nc.dram_tensor("ho", [B_CORE, S_CORE], f16, kind="ExternalOutput")
    co = nc.dram_tensor("co", [B_CORE, S_CORE], f16, kind="ExternalOutput")

    xt_r = xt.ap().rearrange("(kt p) n -> kt p n", p=128)
    wf_r = w["f"].ap().rearrange("(kt p) n -> kt p n", p=128)
    wi_src = w["i"].ap().rearrange("(ch kt p) n -> ch p kt n",
                                   p=128, kt=KT // WCH)
    wo_src = w["o"].ap().rearrange("(kt p) n -> p kt n", p=128)
    c0_src = c0.ap().rearrange("(bt p) n -> p bt n", p=128)
    ho_r = ho.ap().rearrange("(bt p) n -> bt p n", p=128)
    co_r = co.ap().rearrange("(bt p) n -> bt p n", p=128)

    SIG = mybir.ActivationFunctionType.Sigmoid
    TANH = mybir.ActivationFunctionType.Tanh

    with tile.TileContext(nc) as tc:
        with (
            tc.tile_pool(name="xtp", bufs=KT) as xtp,
            tc.tile_pool(name="wfp", bufs=KT) as wfp,
            tc.tile_pool(name="wip", bufs=WCH) as wip,
            tc.tile_pool(name="bigp", bufs=1) as bigp,
            tc.tile_pool(name="sigfp", bufs=BT) as sigfp,
            tc.tile_pool(name="t1p", bufs=BT) as t1p,
            tc.tile_pool(name="workp", bufs=3) as workp,
            tc.tile_pool(name="psp", bufs=8, space="PSUM") as psp,
        ):
            # ---- PE warm-up: a zero k-step. 8 matmuls on a memset tile
            # accumulate zeros into the real phase-f PSUM banks (start=True;
            # the real k0 uses start=False). They run during the initial DMA
            # wait, form one continuous PE stream with phase f (no idle gap
            # can reset the HAM busy-window), and pin the warm-up cost to
            # the otherwise-idle span. ----
            dummy = bigp.tile([128, S_CORE], f16, name="dummy", tag="dummy")
            nc.vector.memset(dummy[:], 0.0)

            # ---- input DMAs ----
            # scalar ring: the wf stream
            wf_tiles = []
            for k in range(KT):
                wt = wfp.tile([128, S_CORE], f16, name=f"wf_{k}", tag="wf")
                nc.scalar.dma_start(out=wt[:], in_=wf_r[k])
                wf_tiles.append(wt)
            # sync ring: xt stream, then phase i/o tensors in first-use order
            xt_tiles = []
            for k in range(KT):
                xtt = xtp.tile([128, B_CORE], f16, name=f"xt_{k}", tag="xt")
                if k == 0:
                    h = B_CORE // 2
                    nc.sync.dma_start(out=xtt[:, :h], in_=xt_r[k][:, :h])
                    nc.sync.dma_start(out=xtt[:, h:], in_=xt_r[k][:, h:])
                else:
                    nc.sync.dma_start(out=xtt[:], in_=xt_r[k])
                xt_tiles.append(xtt)
            bias_sb = {}
            bias_sb["f"] = bigp.tile([128, S_CORE], f32, name="bf_sb",
                                     tag="bf")
            nc.sync.dma_start(out=bias_sb["f"][:], in_=bias["f"].ap())
            wi_tiles = []
            for ch in range(WCH):
                wic = wip.tile([128, KT // WCH, S_CORE], f16,
                               name=f"wi_{ch}", tag="wi")
                nc.sync.dma_start(out=wic[:], in_=wi_src[ch])
                wi_tiles.append(wic)
            c0_sb = bigp.tile([128, BT, S_CORE], f16, name="c0_sb", tag="c0")
            nc.sync.dma_start(out=c0_sb[:], in_=c0_src)
            bias_sb["i"] = bigp.tile([128, S_CORE], f32, name="bi_sb",
                                     tag="bi")
            nc.sync.dma_start(out=bias_sb["i"][:], in_=bias["i"].ap())
            wo_sb = bigp.tile([128, KT, S_CORE], f16, name="wo_sb", tag="wo")
            nc.sync.dma_start(out=wo_sb[:], in_=wo_src)
            bias_sb["o"] = bigp.tile([128, S_CORE], f32, name="bo_sb",
                                     tag="bo")
            nc.sync.dma_start(out=bias_sb["o"][:], in_=bias["o"].ap())

            # ---- phase f: k-major over all 8 PSUM banks ----
            ps_f = [psp.tile([128, S_CORE], f32, name=f"psf_{bt}", tag="ps")
                    for bt in range(BT)]
            for bt in range(BT):
                nc.tensor.matmul(ps_f[bt][:], dummy[:, :128], dummy[:],
                                 start=True, stop=False)
            for k in range(KT):
                for bt in range(BT):
                    nc.tensor.matmul(
                        ps_f[bt][:],
                        xt_tiles[k][:, bt * 128:(bt + 1) * 128],
                        wf_tiles[k][:],
                        start=False, stop=(k == KT - 1))
            sig_f = []
            for bt in range(BT):
                nc.vector.tensor_add(ps_f[bt][:], ps_f[bt][:],
                                     bias_sb["f"][:])
                sf = sigfp.tile([128, S_CORE], f16, name=f"sigf_{bt}",
                                tag="sigf")
                nc.scalar.activation(sf[:], ps_f[bt][:], SIG)
                sig_f.append(sf)

            # ---- phase i: two 4-bank groups, k-major within each, so the
            # group-A elementwise overlaps group-B matmuls ----
            for grp in range(2):
                bts = range(grp * 4, grp * 4 + 4)
                ps_i = {bt: psp.tile([128, S_CORE], f32, name=f"psi_{bt}",
                                     tag="ps") for bt in bts}
                for k in range(KT):
                    for bt in bts:
                        nc.tensor.matmul(
                            ps_i[bt][:],
                            xt_tiles[k][:, bt * 128:(bt + 1) * 128],
                            wi_tiles[k // (KT // WCH)][:, k % (KT // WCH), :],
                            start=(k == 0), stop=(k == KT - 1))
                for bt in bts:
                    nc.vector.tensor_add(ps_i[bt][:], ps_i[bt][:],
                                         bias_sb["i"][:])
                    t1 = t1p.tile([128, S_CORE], f16, name=f"t1_{bt}",
                                  tag="t1")
                    nc.scalar.activation(t1[:], ps_i[bt][:], SIG)
                    nc.vector.tensor_add(t1[:], t1[:], c0_sb[:, bt, :])
                    cn = workp.tile([128, S_CORE], f16, name=f"cn_{bt}",
                                    tag="cn")
                    nc.vector.tensor_mul(cn[:], sig_f[bt][:], t1[:])
                    # tanh overwrites the sig_f slot (sig_f consumed by cn)
                    nc.scalar.activation(sig_f[bt][:], cn[:], TANH)
                    nc.scalar.dma_start(out=co_r[bt], in_=cn[:])

            # ---- phase o: bt-major; hn = sig_o * tanh(cn), store ho ----
            for bt in range(BT):
                so = workp.tile([128, S_CORE], f16, name=f"so_{bt}", tag="so")
                hn = workp.tile([128, S_CORE], f16, name=f"hn_{bt}", tag="hn")
                if bt < BT - 1:
                    ps = psp.tile([128, S_CORE], f32, name=f"pso_{bt}",
                                  tag="ps")
                    for k in range(KT):
                        nc.tensor.matmul(
                            ps[:],
                            xt_tiles[k][:, bt * 128:(bt + 1) * 128],
                            wo_sb[:, k, :],
                            start=(k == 0), stop=(k == KT - 1))
                    nc.vector.tensor_add(ps[:], ps[:], bias_sb["o"][:])
                    nc.scalar.activation(so[:], ps[:], SIG)
                    nc.vector.tensor_mul(hn[:], so[:], sig_f[bt][:])
                    nc.scalar.dma_start(out=ho_r[bt], in_=hn[:])
                else:
                    # final tile: three column-chunks, each with its own
                    # accumulation, so earlier chunks' chains overlap later
                    # chunks' matmuls and only a 128-col chain trails
                    chunks = [(slice(0, 256), nc.sync),
                              (slice(256, 384), nc.scalar),
                              (slice(384, 512), nc.sync)]
                    for ci, (cs, eng) in enumerate(chunks):
                        psh = psp.tile([128, cs.stop - cs.start], f32,
                                       name=f"pso7_{ci}", tag="ps")
                        for k in range(KT):
                            nc.tensor.matmul(
                                psh[:],
                                xt_tiles[k][:, bt * 128:(bt + 1) * 128],
                                wo_sb[:, k, cs],
                                start=(k == 0), stop=(k == KT - 1))
                        nc.vector.tensor_add(psh[:], psh[:],
                                             bias_sb["o"][:, cs])
                        nc.scalar.activation(so[:, cs], psh[:], SIG)
                        nc.vector.tensor_mul(hn[:, cs], so[:, cs],
                                             sig_f[bt][:, cs])
                        eng.dma_start(out=ho_r[bt][:, cs], in_=hn[:, cs])

    nc.compile()
    return nc


def _get_nc():
    if "nc" not in _CACHE:
        _CACHE["nc"] = _build_nc()
    return _CACHE["nc"]


def _prep_in_maps(inputs, h0, c0, ws, us, bs):
    """ws/us/bs: dicts g -> full array."""
    in_maps = []
    xts = []
    for blk in range(BB):
        rows = slice(blk * B_CORE, (blk + 1) * B_CORE)
        x = np.concatenate([inputs[rows], h0[rows]], axis=1)  # [1024, 2048]
        xts.append(np.ascontiguousarray(x.T).astype(np.float16))  # [2048, 1024]
    wgs = {}
    biases = {}
    for g in "fio":
        for sb in range(SB):
            cols = slice(sb * S_CORE, (sb + 1) * S_CORE)
            wgs[(g, sb)] = np.ascontiguousarray(
                np.concatenate([ws[g][:, cols], us[g][:, cols]],
                               axis=0)).astype(np.float16)
            biases[(g, sb)] = np.ascontiguousarray(
                np.broadcast_to(bs[g][cols], (128, S_CORE)).astype(
                    np.float32))
    for core in range(N_CORES):
        blk, sb = core // SB, core % SB
        rows = slice(blk * B_CORE, (blk + 1) * B_CORE)
        cols = slice(sb * S_CORE, (sb + 1) * S_CORE)
        m = {"xt": xts[blk],
             "c0": np.ascontiguousarray(c0[rows, cols]).astype(np.float16)}
        for g in "fio":
            m[f"w{g}"] = wgs[(g, sb)]
            m[f"b{g}"] = biases[(g, sb)]
        in_maps.append(m)
    return in_maps


def _run(in_maps, trace=False, trace_kwargs=None, tmpdir=None):
    from concourse.bass_utils import run_bass_kernel_spmd

    nc = _get_nc()
    return run_bass_kernel_spmd(
        nc, in_maps, list(range(N_CORES)), trace=trace,
        trace_kwargs=trace_kwargs or {}, tmpdir=tmpdir,
    )


def _assemble(results):
    h = np.empty((B, S), dtype=np.float32)
    c = np.empty((B, S), dtype=np.float32)
    for core in range(N_CORES):
        blk, sb = core // SB, core % SB
        rows = slice(blk * B_CORE, (blk + 1) * B_CORE)
        cols = slice(sb * S_CORE, (sb + 1) * S_CORE)
        h[rows, cols] = results[core]["ho"].astype(np.float32)
        c[rows, cols] = results[core]["co"].astype(np.float32)
    return h, c


def kernel(inputs, h0, c0, w_f, u_f, b_f, w_i, u_i, b_i, w_o, u_o, b_o):
    inputs = np.asarray(inputs, dtype=np.float32)
    h0 = np.asarray(h0, dtype=np.float32)
    c0 = np.asarray(c0, dtype=np.float32)
    ws = {"f": np.asarray(w_f, np.float32), "i": np.asarray(w_i, np.float32),
          "o": np.asarray(w_o, np.float32)}
    us = {"f": np.asarray(u_f, np.float32), "i": np.asarray(u_i, np.float32),
          "o": np.asarray(u_o, np.float32)}
    bs = {"f": np.asarray(b_f, np.float32), "i": np.asarray(b_i, np.float32),
          "o": np.asarray(b_o, np.float32)}
    in_maps = _prep_in_maps(inputs, h0, c0, ws, us, bs)
    res = _run(in_maps)
    return _assemble(res.results)


# revision 13
# speedup vs baseline: 1.0016x; 1.0016x over previous
"""Trainium2 Bass kernel for nn_BasicLSTM (single-step LSTM cell variant).

Reference computation (B=4096, D=1024, S=1024):
    pre_f = inputs @ w_f + h0 @ u_f + b_f
    f     = sigmoid(pre_f)
    i     = sigmoid(inputs @ w_i + h0 @ u_i + b_i)
    o     = sigmoid(inputs @ w_o + h0 @ u_o + b_o)
    c_new = f * c0 + f * i          (input_cell reuses the forget gate)
    h_new = o * tanh(c_new)
    returns (h_new, c_new)

Sharding: batch 4-way x state 2-way over 8 NeuronCores. Core c handles
batch rows [ (c//2)*1024 : (c//2+1)*1024 ) and state cols
[ (c%2)*512 : (c%2+1)*512 ). Host-side prep per core:
    xt  = concat([inputs_rows, h0_rows], 1).T           # [2048, 1024] fp16
    w_g = concat([w_g[:, cols], u_g[:, cols]], 0)       # [2048, 512]  fp16
so the device kernel is three plain matmuls (K=2048 contraction on the
partition axis) plus fused elementwise, no on-device transposes.

Performance notes (the 384-matmul stream runs at the PE roofline of
~216 ns per N=512 fp16 matmul; everything else is arranged around it):
  - DMA instructions cost ~0.6 us of issue time each and the two HWDGE
    rings (sync, scalar) round-robin SDMA bandwidth per packet. The
    phase-f stream is split: xt tiles (2 KB rows) on sync, wf tiles
    (1 KB rows) on scalar -- the 2:1 byte ratio matches the per-packet
    round-robin so each (xt_k, wf_k) pair lands together, ahead of the
    PE's 1.73 us/k-step consumption.
  - Remaining inputs ride the sync ring behind the xt stream, ordered
    by first use: b_f, wi (4 chunks, consumed k-major), c0, b_i, wo,
    b_o. Phases f and i are k-major so weight chunks stream.
  - The HAM clock gate keeps the PE at 1.2 GHz until ~3.4 us of busy;
    four dummy matmuls on a memset tile fill the initial DMA wait.
  - Bias adds run on GpSimd, sig/tanh on Scalar, muls on Vector, so no
    engine queue serializes the phase-boundary chains.
  - c_new is independent of the o-gate: co is computed and stored while
    phase o runs. The tail after the last matmul is bias+sig+mul+store
    of one h tile, split in halves across the two DMA rings.
  - c0 / ho / co move as fp16 (host converts), halving that traffic.
"""

import sys

sys.path.insert(0, "/opt/trn_rl_repo")

import numpy as np

B, D, S = 4096, 1024, 1024
N_CORES = 8
BB, SB = 4, 2          # batch blocks x state blocks
B_CORE = B // BB       # 1024 rows per core
S_CORE = S // SB       # 512 state cols per core
K = D + S              # 2048 contraction
KT = K // 128          # 16 k-tiles
BT = B_CORE // 128     # 8 batch tiles per core
WCH = 4                # wi DMA chunks (k-tiles each)

_CACHE: dict = {}


def _build_nc():
    import concourse.mybir as mybir
    import concourse.tile as tile
    from concourse import bacc

    f32 = mybir.dt.float32
    f16 = mybir.dt.float16

    nc = bacc.Bacc("TRN2", target_bir_lowering=False, debug=False,
                   num_devices=N_CORES)

    xt = nc.dram_tensor("xt", [K, B_CORE], f16, kind="ExternalInput")
    w = {g: nc.dram_tensor(f"w{g}", [K, S_CORE], f16, kind="ExternalInput")
         for g in "fio"}
    bias = {g: nc.dram_tensor(f"b{g}", [128, S_CORE], f32,
                              kind="ExternalInput") for g in "fio"}
    c0 = nc.dram_tensor("c0", [B_CORE, S_CORE], f16, kind="ExternalInput")
    ho = nc.dram_tensor("ho", [B_CORE, S_CORE], f16, kind="ExternalOutput")
    co = nc.dram_tensor("co", [B_CORE, S_CORE], f16, kind="ExternalOutput")

    xt_r = xt.ap().rearrange("(kt p) n -> kt p n", p=128)
    wf_r = w["f"].ap().rearrange("(kt p) n -> kt p n", p=128)
    wi_src = w["i"].ap().rearrange("(ch kt p) n -> ch p kt n",
                                   p=128, kt=KT // WCH)
    wo_src = w["o"].ap().rearrange("(kt p) n -> p kt n", p=128)
    c0_src = c0.ap().rearrange("(bt p) n -> p bt n", p=128)
    ho_r = ho.ap().rearrange("(bt p) n -> bt p n", p=128)
    co_r = co.ap().rearrange("(bt p) n -> bt p n", p=128)

    SIG = mybir.ActivationFunctionType.Sigmoid
    TANH = mybir.ActivationFunctionType.Tanh

    with tile.TileContext(nc) as tc:
        with (
            tc.tile_pool(name="xtp", bufs=KT) as xtp,
            tc.tile_pool(name="wfp", bufs=KT) as wfp,
            tc.tile_pool(name="wip", bufs=WCH) as wip,
            tc.tile_pool(name="bigp", bufs=1) as bigp,
            tc.tile_pool(name="sigfp", bufs=BT) as sigfp,
            tc.tile_pool(name="t1p", bufs=BT) as t1p,
            tc.tile_pool(name="workp", bufs=3) as workp,
            tc.tile_pool(name="psp", bufs=8, space="PSUM") as psp,
        ):
            # ---- PE warm-up: a zero k-step. 8 matmuls on a memset tile
            # accumulate zeros into the real phase-f PSUM banks (start=True;
            # the real k0 uses start=False). They run during the initial DMA
            # wait, form one continuous PE stream with phase f (no idle gap
            # can reset the HAM busy-window), and pin the warm-up cost to
            # the otherwise-idle span. ----
            dummy = bigp.tile([128, S_CORE], f16, name="dummy", tag="dummy")
            nc.vector.memset(dummy[:], 0.0)

            # ---- input DMAs ----
            # scalar ring: the wf stream
            wf_tiles = []
            for k in range(KT):
                wt = wfp.tile([128, S_CORE], f16, name=f"wf_{k}", tag="wf")
                nc.scalar.dma_start(out=wt[:], in_=wf_r[k])
                wf_tiles.append(wt)
            # sync ring: xt stream, then phase i/o tensors in first-use order
            xt_tiles = []
            for k in range(KT):
                xtt = xtp.tile([128, B_CORE], f16, name=f"xt_{k}", tag="xt")
                if k == 0:
                    h = B_CORE // 2
                    nc.sync.dma_start(out=xtt[:, :h], in_=xt_r[k][:, :h])
                    nc.sync.dma_start(out=xtt[:, h:], in_=xt_r[k][:, h:])
                else:
                    nc.sync.dma_start(out=xtt[:], in_=xt_r[k])
                xt_tiles.append(xtt)
            bias_sb = {}
            bias_sb["f"] = bigp.tile([128, S_CORE], f32, name="bf_sb",
                                     tag="bf")
            nc.sync.dma_start(out=bias_sb["f"][:], in_=bias["f"].ap())
            wi_tiles = []
            for ch in range(WCH):
                wic = wip.tile([128, KT // WCH, S_CORE], f16,
                               name=f"wi_{ch}", tag="wi")
                nc.sync.dma_start(out=wic[:], in_=wi_src[ch])
                wi_tiles.append(wic)
            c0_sb = bigp.tile([128, BT, S_CORE], f16, name="c0_sb", tag="c0")
            nc.sync.dma_start(out=c0_sb[:], in_=c0_src)
            bias_sb["i"] = bigp.tile([128, S_CORE], f32, name="bi_sb",
                                     tag="bi")
            nc.sync.dma_start(out=bias_sb["i"][:], in_=bias["i"].ap())
            wo_sb = bigp.tile([128, KT, S_CORE], f16, name="wo_sb", tag="wo")
            nc.sync.dma_start(out=wo_sb[:], in_=wo_src)
            bias_sb["o"] = bigp.tile([128, S_CORE], f32, name="bo_sb",
                                     tag="bo")
            nc.sync.dma_start(out=bias_sb["o"][:], in_=bias["o"].ap())

            # ---- phase f: k-major over all 8 PSUM banks ----
            ps_f = [psp.tile([128, S_CORE], f32, name=f"psf_{bt}", tag="ps")
                    for bt in range(BT)]
            for bt in range(BT):
                nc.tensor.matmul(ps_f[bt][:], dummy[:, :128], dummy[:],
                                 start=True, stop=False)
            for k in range(KT):
                for bt in range(BT):
                    nc.tensor.matmul(
                        ps_f[bt][:],
                        xt_tiles[k][:, bt * 128:(bt + 1) * 128],
                        wf_tiles[k][:],
                        start=False, stop=(k == KT - 1))
            sig_f = []
            for bt in range(BT):
                nc.vector.tensor_add(ps_f[bt][:], ps_f[bt][:],
                                     bias_sb["f"][:])
                sf = sigfp.tile([128, S_CORE], f16, name=f"sigf_{bt}",
                                tag="sigf")
                nc.scalar.activation(sf[:], ps_f[bt][:], SIG)
                sig_f.append(sf)

            # ---- phase i: two 4-bank groups, k-major within each, so the
            # group-A elementwise overlaps group-B matmuls ----
            for grp in range(2):
                bts = range(grp * 4, grp * 4 + 4)
                ps_i = {bt: psp.tile([128, S_CORE], f32, name=f"psi_{bt}",
                                     tag="ps") for bt in bts}
                for k in range(KT):
                    for bt in bts:
                        nc.tensor.matmul(
                            ps_i[bt][:],
                            xt_tiles[k][:, bt * 128:(bt + 1) * 128],
                            wi_tiles[k // (KT // WCH)][:, k % (KT // WCH), :],
                            start=(k == 0), stop=(k == KT - 1))
                for bt in bts:
                    nc.vector.tensor_add(ps_i[bt][:], ps_i[bt][:],
                                         bias_sb["i"][:])
                    t1 = t1p.tile([128, S_CORE], f16, name=f"t1_{bt}",
                                  tag="t1")
                    nc.scalar.activation(t1[:], ps_i[bt][:], SIG)
                    nc.vector.tensor_add(t1[:], t1[:], c0_sb[:, bt, :])
                    cn = workp.tile([128, S_CORE], f16, name=f"cn_{bt}",
                                    tag="cn")
                    nc.vector.tensor_mul(cn[:], sig_f[bt][:], t1[:])
                    # tanh overwrites the sig_f slot (sig_f consumed by cn)
                    nc.scalar.activation(sig_f[bt][:], cn[:], TANH)
                    nc.scalar.dma_start(out=co_r[bt], in_=cn[:])

            # ---- phase o: bt-major; hn = sig_o * tanh(cn), store ho ----
            for bt in range(BT):
                so = workp.tile([128, S_CORE], f16, name=f"so_{bt}", tag="so")
                hn = workp.tile([128, S_CORE], f16, name=f"hn_{bt}", tag="hn")
                if bt < BT - 1:
                    ps = psp.tile([128, S_CORE], f32, name=f"pso_{bt}",
                                  tag="ps")
                    for k in range(KT):
                        nc.tensor.matmul(
                            ps[:],
                            xt_tiles[k][:, bt * 128:(bt + 1) * 128],
                            wo_sb[:, k, :],
                            start=(k == 0), stop=(k == KT - 1))
                    nc.vector.tensor_add(ps[:], ps[:], bias_sb["o"][:])
                    nc.scalar.activation(so[:], ps[:], SIG)
                    nc.vector.tensor_mul(hn[:], so[:], sig_f[bt][:])
                    nc.scalar.dma_start(out=ho_r[bt], in_=hn[:])
                else:
                    # final tile: three column-chunks, each with its own
                    # accumulation, so earlier chunks' chains overlap later
                    # chunks' matmuls and only a 128-col chain trails
                    chunks = [(slice(0, 256), nc.sync),
                              (slice(256, 384), nc.sync),
                              (slice(384, 512), nc.scalar)]
                    for ci, (cs, eng) in enumerate(chunks):
                        psh = psp.tile([128, cs.stop - cs.start], f32,
                                       name=f"pso7_{ci}", tag="ps")
                        for k in range(KT):
                            nc.tensor.matmul(
                                psh[:],
                                xt_tiles[k][:, bt * 128:(bt + 1) * 128],
                                wo_sb[:, k, cs],
                                start=(k == 0), stop=(k == KT - 1))
                        nc.vector.tensor_add(psh[:], psh[:],
                                             bias_sb["o"][:, cs])
                        nc.scalar.activation(so[:, cs], psh[:], SIG)
                        nc.vector.tensor_mul(hn[:, cs], so[:, cs],
                                             sig_f[bt][:, cs])
                        eng.dma_start(out=ho_r[bt][:, cs], in_=hn[:, cs])

    nc.compile()
    return nc


def _get_nc():
    if "nc" not in _CACHE:
        _CACHE["nc"] = _build_nc()
    return _CACHE["nc"]


def _prep_in_maps(inputs, h0, c0, ws, us, bs):
    """ws/us/bs: dicts g -> full array."""
    in_maps = []
    xts = []
    for blk in range(BB):
        rows = slice(blk * B_CORE, (blk + 1) * B_CORE)
        x = np.concatenate([inputs[rows], h0[rows]], axis=1)  # [1024, 2048]
        xts.append(np.ascontiguousarray(x.T).astype(np.float16))  # [2048, 1024]
    wgs = {}
    biases = {}
    for g in "fio":
        for sb in range(SB):
            cols = slice(sb * S_CORE, (sb + 1) * S_CORE)
            wgs[(g, sb)] = np.ascontiguousarray(
                np.concatenate([ws[g][:, cols], us[g][:, cols]],
                               axis=0)).astype(np.float16)
            biases[(g, sb)] = np.ascontiguousarray(
                np.broadcast_to(bs[g][cols], (128, S_CORE)).astype(
                    np.float32))
    for core in range(N_CORES):
        blk, sb = core // SB, core % SB
        rows = slice(blk * B_CORE, (blk + 1) * B_CORE)
        cols = slice(sb * S_CORE, (sb + 1) * S_CORE)
        m = {"xt": xts[blk],
             "c0": np.ascontiguousarray(c0[rows, cols]).astype(np.float16)}
        for g in "fio":
            m[f"w{g}"] = wgs[(g, sb)]
            m[f"b{g}"] = biases[(g, sb)]
        in_maps.append(m)
    return in_maps


def _run(in_maps, trace=False, trace_kwargs=None, tmpdir=None):
    from concourse.bass_utils import run_bass_kernel_spmd

    nc = _get_nc()
    return run_bass_kernel_spmd(
        nc, in_maps, list(range(N_CORES)), trace=trace,
        trace_kwargs=trace_kwargs or {}, tmpdir=tmpdir,
    )


def _assemble(results):
    h = np.empty((B, S), dtype=np.float32)
    c = np.empty((B, S), dtype=np.float32)
    for core in range(N_CORES):
        blk, sb = core // SB, core % SB
        rows = slice(blk * B_CORE, (blk + 1) * B_CORE)
        cols = slice(sb * S_CORE, (sb + 1) * S_CORE)
        h[rows, cols] = results[core]["ho"].astype(np.float32)
        c[rows, cols] = results[core]["co"].astype(np.float32)
    return h, c


def kernel(inputs, h0, c0, w_f, u_f, b_f, w_i, u_i, b_i, w_o, u_o, b_o):
    inputs = np.asarray(inputs, dtype=np.float32)
    h0 = np.asarray(h0, dtype=np.float32)
    c0 = np.asarray(c0, dtype=np.float32)
    ws = {"f": np.asarray(w_f, np.float32), "i": np.asarray(w_i, np.float32),
          "o": np.asarray(w_o, np.float32)}
    us = {"f": np.asarray(u_f, np.float32), "i": np.asarray(u_i, np.float32),
          "o": np.asarray(u_o, np.float32)}
    bs = {"f": np.asarray(b_f, np.float32), "i": np.asarray(b_i, np.float32),
          "o": np.asarray(b_o, np.float32)}
    in_maps = _prep_in_maps(inputs, h0, c0, ws, us, bs)
    res = _run(in_maps)
    return _assemble(res.results)


# revision 16
# speedup vs baseline: 1.0089x; 1.0072x over previous
"""Trainium2 Bass kernel for nn_BasicLSTM (single-step LSTM cell variant).

Reference computation (B=4096, D=1024, S=1024):
    pre_f = inputs @ w_f + h0 @ u_f + b_f
    f     = sigmoid(pre_f)
    i     = sigmoid(inputs @ w_i + h0 @ u_i + b_i)
    o     = sigmoid(inputs @ w_o + h0 @ u_o + b_o)
    c_new = f * c0 + f * i          (input_cell reuses the forget gate)
    h_new = o * tanh(c_new)
    returns (h_new, c_new)

Sharding: batch 4-way x state 2-way over 8 NeuronCores. Core c handles
batch rows [ (c//2)*1024 : (c//2+1)*1024 ) and state cols
[ (c%2)*512 : (c%2+1)*512 ). Host-side prep per core:
    xt  = concat([inputs_rows, h0_rows], 1).T           # [2048, 1024] fp16
    w_g = concat([w_g[:, cols], u_g[:, cols]], 0)       # [2048, 512]  fp16
so the device kernel is three plain matmuls (K=2048 contraction on the
partition axis) plus fused elementwise, no on-device transposes.

Performance notes (the 384-matmul stream runs at the PE roofline of
~216 ns per N=512 fp16 matmul; everything else is arranged around it):
  - DMA instructions cost ~0.6 us of issue time each and the two HWDGE
    rings (sync, scalar) round-robin SDMA bandwidth per packet. The
    phase-f stream is split: xt tiles (2 KB rows) on sync, wf tiles
    (1 KB rows) on scalar -- the 2:1 byte ratio matches the per-packet
    round-robin so each (xt_k, wf_k) pair lands together, ahead of the
    PE's 1.73 us/k-step consumption.
  - Remaining inputs ride the sync ring behind the xt stream, ordered
    by first use: b_f, wi (4 chunks, consumed k-major), c0, b_i, wo,
    b_o. Phases f and i are k-major so weight chunks stream.
  - The HAM clock gate keeps the PE at 1.2 GHz until ~3.4 us of busy;
    four dummy matmuls on a memset tile fill the initial DMA wait.
  - Bias adds run on GpSimd, sig/tanh on Scalar, muls on Vector, so no
    engine queue serializes the phase-boundary chains.
  - c_new is independent of the o-gate: co is computed and stored while
    phase o runs. The tail after the last matmul is bias+sig+mul+store
    of one h tile, split in halves across the two DMA rings.
  - c0 / ho / co move as fp16 (host converts), halving that traffic.
"""

import sys

sys.path.insert(0, "/opt/trn_rl_repo")

import numpy as np

B, D, S = 4096, 1024, 1024
N_CORES = 8
BB, SB = 4, 2          # batch blocks x state blocks
B_CORE = B // BB       # 1024 rows per core
S_CORE = S // SB       # 512 state cols per core
K = D + S              # 2048 contraction
KT = K // 128          # 16 k-tiles
BT = B_CORE // 128     # 8 batch tiles per core
WCH = 4                # wi DMA chunks (k-tiles each)

_CACHE: dict = {}


def _build_nc():
    import concourse.mybir as mybir
    import concourse.tile as tile
    from concourse import bacc

    f32 = mybir.dt.float32
    f16 = mybir.dt.float16

    nc = bacc.Bacc("TRN2", target_bir_lowering=False, debug=False,
                   num_devices=N_CORES)

    xt = nc.dram_tensor("xt", [K, B_CORE], f16, kind="ExternalInput")
    w = {g: nc.dram_tensor(f"w{g}", [K, S_CORE], f16, kind="ExternalInput")
         for g in "fio"}
    bias = {g: nc.dram_tensor(f"b{g}", [128, S_CORE], f32,
                              kind="ExternalInput") for g in "fio"}
    c0 = nc.dram_tensor("c0", [B_CORE, S_CORE], f16, kind="ExternalInput")
    ho = nc.dram_tensor("ho", [B_CORE, S_CORE], f16, kind="ExternalOutput")
    co = nc.dram_tensor("co", [B_CORE, S_CORE], f16, kind="ExternalOutput")

    xt_r = xt.ap().rearrange("(kt p) n -> kt p n", p=128)
    wf_r = w["f"].ap().rearrange("(kt p) n -> kt p n", p=128)
    wi_src = w["i"].ap().rearrange("(ch kt p) n -> ch p kt n",
                                   p=128, kt=KT // WCH)
    wo_src = w["o"].ap().rearrange("(kt p) n -> p kt n", p=128)
    c0_src = c0.ap().rearrange("(bt p) n -> p bt n", p=128)
    ho_r = ho.ap().rearrange("(bt p) n -> bt p n", p=128)
    co_r = co.ap().rearrange("(bt p) n -> bt p n", p=128)

    SIG = mybir.ActivationFunctionType.Sigmoid
    TANH = mybir.ActivationFunctionType.Tanh

    with tile.TileContext(nc) as tc:
        with (
            tc.tile_pool(name="xtp", bufs=KT) as xtp,
            tc.tile_pool(name="wfp", bufs=KT) as wfp,
            tc.tile_pool(name="wip", bufs=WCH) as wip,
            tc.tile_pool(name="bigp", bufs=1) as bigp,
            tc.tile_pool(name="sigfp", bufs=BT) as sigfp,
            tc.tile_pool(name="t1p", bufs=BT) as t1p,
            tc.tile_pool(name="workp", bufs=3) as workp,
            tc.tile_pool(name="psp", bufs=8, space="PSUM") as psp,
        ):
            # ---- PE warm-up: a ones k-step. 8 matmuls over the framework's
            # bf16-1.0 const region (0-stride broadcast APs -- no DMA or
            # memset dependency, so they start the moment the PE queue
            # opens) accumulate into the real phase-f PSUM banks
            # (start=True; the real k0 uses start=False). They run during
            # the initial DMA wait, form one continuous PE stream with
            # phase f (no idle gap can reset the HAM busy-window), and each
            # adds exactly 128.0 per element, which the host cancels by
            # shifting b_f. ----
            bf16 = mybir.dt.bfloat16
            ones_st = nc.const_aps.tensor(1.0, (128, 128), bf16)
            ones_mv = nc.const_aps.tensor(1.0, (128, S_CORE), bf16)

            # ---- input DMAs ----
            # scalar ring: the wf stream
            wf_tiles = []
            for k in range(KT):
                wt = wfp.tile([128, S_CORE], f16, name=f"wf_{k}", tag="wf")
                nc.scalar.dma_start(out=wt[:], in_=wf_r[k])
                wf_tiles.append(wt)
            # sync ring: xt stream, then phase i/o tensors in first-use order
            xt_tiles = []
            for k in range(KT):
                xtt = xtp.tile([128, B_CORE], f16, name=f"xt_{k}", tag="xt")
                if k == 0:
                    h = B_CORE // 2
                    nc.sync.dma_start(out=xtt[:, :h], in_=xt_r[k][:, :h])
                    nc.sync.dma_start(out=xtt[:, h:], in_=xt_r[k][:, h:])
                else:
                    nc.sync.dma_start(out=xtt[:], in_=xt_r[k])
                xt_tiles.append(xtt)
            bias_sb = {}
            bias_sb["f"] = bigp.tile([128, S_CORE], f32, name="bf_sb",
                                     tag="bf")
            nc.sync.dma_start(out=bias_sb["f"][:], in_=bias["f"].ap())
            wi_tiles = []
            for ch in range(WCH):
                wic = wip.tile([128, KT // WCH, S_CORE], f16,
                               name=f"wi_{ch}", tag="wi")
                nc.sync.dma_start(out=wic[:], in_=wi_src[ch])
                wi_tiles.append(wic)
            c0_sb = bigp.tile([128, BT, S_CORE], f16, name="c0_sb", tag="c0")
            nc.sync.dma_start(out=c0_sb[:], in_=c0_src)
            bias_sb["i"] = bigp.tile([128, S_CORE], f32, name="bi_sb",
                                     tag="bi")
            nc.sync.dma_start(out=bias_sb["i"][:], in_=bias["i"].ap())
            wo_sb = bigp.tile([128, KT, S_CORE], f16, name="wo_sb", tag="wo")
            nc.sync.dma_start(out=wo_sb[:], in_=wo_src)
            bias_sb["o"] = bigp.tile([128, S_CORE], f32, name="bo_sb",
                                     tag="bo")
            nc.sync.dma_start(out=bias_sb["o"][:], in_=bias["o"].ap())

            # ---- phase f: k-major over all 8 PSUM banks ----
            ps_f = [psp.tile([128, S_CORE], f32, name=f"psf_{bt}", tag="ps")
                    for bt in range(BT)]
            for bt in range(BT):
                nc.tensor.matmul(ps_f[bt][:], ones_st, ones_mv,
                                 start=True, stop=False)
            for k in range(KT):
                for bt in range(BT):
                    nc.tensor.matmul(
                        ps_f[bt][:],
                        xt_tiles[k][:, bt * 128:(bt + 1) * 128],
                        wf_tiles[k][:],
                        start=False, stop=(k == KT - 1))
            sig_f = []
            for bt in range(BT):
                nc.vector.tensor_add(ps_f[bt][:], ps_f[bt][:],
                                     bias_sb["f"][:])
                sf = sigfp.tile([128, S_CORE], f16, name=f"sigf_{bt}",
                                tag="sigf")
                nc.scalar.activation(sf[:], ps_f[bt][:], SIG)
                sig_f.append(sf)

            # ---- phase i: two 4-bank groups, k-major within each, so the
            # group-A elementwise overlaps group-B matmuls ----
            for grp in range(2):
                bts = range(grp * 4, grp * 4 + 4)
                ps_i = {bt: psp.tile([128, S_CORE], f32, name=f"psi_{bt}",
                                     tag="ps") for bt in bts}
                for k in range(KT):
                    for bt in bts:
                        nc.tensor.matmul(
                            ps_i[bt][:],
                            xt_tiles[k][:, bt * 128:(bt + 1) * 128],
                            wi_tiles[k // (KT // WCH)][:, k % (KT // WCH), :],
                            start=(k == 0), stop=(k == KT - 1))
                for bt in bts:
                    nc.vector.tensor_add(ps_i[bt][:], ps_i[bt][:],
                                         bias_sb["i"][:])
                    t1 = t1p.tile([128, S_CORE], f16, name=f"t1_{bt}",
                                  tag="t1")
                    nc.scalar.activation(t1[:], ps_i[bt][:], SIG)
                    nc.vector.tensor_add(t1[:], t1[:], c0_sb[:, bt, :])
                    cn = workp.tile([128, S_CORE], f16, name=f"cn_{bt}",
                                    tag="cn")
                    nc.vector.tensor_mul(cn[:], sig_f[bt][:], t1[:])
                    # tanh overwrites the sig_f slot (sig_f consumed by cn)
                    nc.scalar.activation(sig_f[bt][:], cn[:], TANH)
                    nc.scalar.dma_start(out=co_r[bt], in_=cn[:])

            # ---- phase o: bt-major; hn = sig_o * tanh(cn), store ho ----
            for bt in range(BT):
                so = workp.tile([128, S_CORE], f16, name=f"so_{bt}", tag="so")
                hn = workp.tile([128, S_CORE], f16, name=f"hn_{bt}", tag="hn")
                if bt < BT - 1:
                    ps = psp.tile([128, S_CORE], f32, name=f"pso_{bt}",
                                  tag="ps")
                    for k in range(KT):
                        nc.tensor.matmul(
                            ps[:],
                            xt_tiles[k][:, bt * 128:(bt + 1) * 128],
                            wo_sb[:, k, :],
                            start=(k == 0), stop=(k == KT - 1))
                    nc.vector.tensor_add(ps[:], ps[:], bias_sb["o"][:])
                    nc.scalar.activation(so[:], ps[:], SIG)
                    nc.vector.tensor_mul(hn[:], so[:], sig_f[bt][:])
                    nc.scalar.dma_start(out=ho_r[bt], in_=hn[:])
                else:
                    # final tile: three column-chunks, each with its own
                    # accumulation, so earlier chunks' chains overlap later
                    # chunks' matmuls and only a 128-col chain trails
                    chunks = [(slice(0, 256), nc.sync),
                              (slice(256, 384), nc.sync),
                              (slice(384, 512), nc.scalar)]
                    for ci, (cs, eng) in enumerate(chunks):
                        psh = psp.tile([128, cs.stop - cs.start], f32,
                                       name=f"pso7_{ci}", tag="ps")
                        for k in range(KT):
                            nc.tensor.matmul(
                                psh[:],
                                xt_tiles[k][:, bt * 128:(bt + 1) * 128],
                                wo_sb[:, k, cs],
                                start=(k == 0), stop=(k == KT - 1))
                        nc.vector.tensor_add(psh[:], psh[:],
                                             bias_sb["o"][:, cs])
                        nc.scalar.activation(so[:, cs], psh[:], SIG)
                        nc.vector.tensor_mul(hn[:, cs], so[:, cs],
                                             sig_f[bt][:, cs])
                        eng.dma_start(out=ho_r[bt][:, cs], in_=hn[:, cs])

    nc.compile()
    return nc


def _get_nc():
    if "nc" not in _CACHE:
        _CACHE["nc"] = _build_nc()
    return _CACHE["nc"]


def _prep_in_maps(inputs, h0, c0, ws, us, bs):
    """ws/us/bs: dicts g -> full array."""
    in_maps = []
    xts = []
    for blk in range(BB):
        rows = slice(blk * B_CORE, (blk + 1) * B_CORE)
        x = np.concatenate([inputs[rows], h0[rows]], axis=1)  # [1024, 2048]
        xts.append(np.ascontiguousarray(x.T).astype(np.float16))  # [2048, 1024]
    wgs = {}
    biases = {}
    for g in "fio":
        for sb in range(SB):
            cols = slice(sb * S_CORE, (sb + 1) * S_CORE)
            wgs[(g, sb)] = np.ascontiguousarray(
                np.concatenate([ws[g][:, cols], us[g][:, cols]],
                               axis=0)).astype(np.float16)
            # the warm-up ones-matmul adds 128.0 to every phase-f PSUM
            # element; cancel it in the forget-gate bias
            boff = -128.0 if g == "f" else 0.0
            biases[(g, sb)] = np.ascontiguousarray(
                np.broadcast_to(bs[g][cols] + boff, (128, S_CORE)).astype(
                    np.float32))
    for core in range(N_CORES):
        blk, sb = core // SB, core % SB
        rows = slice(blk * B_CORE, (blk + 1) * B_CORE)
        cols = slice(sb * S_CORE, (sb + 1) * S_CORE)
        m = {"xt": xts[blk],
             "c0": np.ascontiguousarray(c0[rows, cols]).astype(np.float16)}
        for g in "fio":
            m[f"w{g}"] = wgs[(g, sb)]
            m[f"b{g}"] = biases[(g, sb)]
        in_maps.append(m)
    return in_maps


def _run(in_maps, trace=False, trace_kwargs=None, tmpdir=None):
    from concourse.bass_utils import run_bass_kernel_spmd

    nc = _get_nc()
    return run_bass_kernel_spmd(
        nc, in_maps, list(range(N_CORES)), trace=trace,
        trace_kwargs=trace_kwargs or {}, tmpdir=tmpdir,
    )


def _assemble(results):
    h = np.empty((B, S), dtype=np.float32)
    c = np.empty((B, S), dtype=np.float32)
    for core in range(N_CORES):
        blk, sb = core // SB, core % SB
        rows = slice(blk * B_CORE, (blk + 1) * B_CORE)
        cols = slice(sb * S_CORE, (sb + 1) * S_CORE)
        h[rows, cols] = results[core]["ho"].astype(np.float32)
        c[rows, cols] = results[core]["co"].astype(np.float32)
    return h, c


def kernel(inputs, h0, c0, w_f, u_f, b_f, w_i, u_i, b_i, w_o, u_o, b_o):
    inputs = np.asarray(inputs, dtype=np.float32)
    h0 = np.asarray(h0, dtype=np.float32)
    c0 = np.asarray(c0, dtype=np.float32)
    ws = {"f": np.asarray(w_f, np.float32), "i": np.asarray(w_i, np.float32),
          "o": np.asarray(w_o, np.float32)}
    us = {"f": np.asarray(u_f, np.float32), "i": np.asarray(u_i, np.float32),
          "o": np.asarray(u_o, np.float32)}
    bs = {"f": np.asarray(b_f, np.float32), "i": np.asarray(b_i, np.float32),
          "o": np.asarray(b_o, np.float32)}
    in_maps = _prep_in_maps(inputs, h0, c0, ws, us, bs)
    res = _run(in_maps)
    return _assemble(res.results)
